# revision 5
# baseline (speedup 1.0000x reference)
"""Trainium2 Bass kernel for nn_DiffuseShader.

Math restructuring (validated against the jax reference to ~1 flip in 13.1M
mask elements):

The reference computes, per point pp (P=512), face nn (N=1024), ray ss (S=25):
  t        ray/plane distance, but read through a torch-style .view(P,N,S) of
           the natural [P*S, N] buffer -> t[pp,nn,ss] = tnat[pp] flattened at
           j = nn*S+ss, i.e. tnat[pp, j//N, j%N].
  gam/beta/alpha   barycentric quantities, each a LINEAR functional of
           r = o + t*d:  gam = wg.r, beta = bw.r, s'' = sw.r  (per-face fp64
           folded weights, invden/invD/sign folded in).
  mask = [gam>0 & beta>0 & s''>0 & t>-1e-4 & t<1 & ~empty]  as 0.0/1.0.

Sharding: points across the 8 cores (64 each), embarrassingly parallel.

Per core device pipeline:
  natural phase:  vd[n', (pp,ss)] = nrm.d  via PE (K=3 matmuls), t = num * 1/vd
                  (custom-DVE approx reciprocal, ~2 ULP), DMA to a DRAM scratch
                  tbuf[pp, ss*N+n'] (the flat-j buffer).
  output phase:   re-read tbuf with the [nn,ss]-strided AP (the .view scramble
                  is free: same flat buffer, different strides), PE computes the
                  three weight planes Wd/Cd/Dd = (weights x d), DVE/GPSIMD/ACT
                  run the 9-op decision chain, DMA the mask out.
"""
import numpy as np

P, N, S, M = 512, 1024, 25, 8
NCORES = 8
PC = P // NCORES          # 64 points per core
FT = N // 128             # 8 face tiles
SUB = 4                   # 400-wide psum subchunks per 1600-wide chunk
W = PC * S // SUB         # 400
NEG_BIG = np.float32(-1e30)

_cache = {}


def _build_module():
    import concourse.bass as bass
    import concourse.tile as tile
    from concourse import bacc, mybir

    f32 = mybir.dt.float32
    Alu = mybir.AluOpType
    Act = mybir.ActivationFunctionType

    nc = bacc.Bacc("TRN2", target_bir_lowering=False, debug=False,
                   num_devices=NCORES)

    dstk_d = nc.dram_tensor("dstk", [3, PC * S], f32, kind="ExternalInput")
    nrmT_d = nc.dram_tensor("nrmT", [3, N], f32, kind="ExternalInput")
    numt_d = nc.dram_tensor("numt", [N, PC], f32, kind="ExternalInput")
    wgT_d = nc.dram_tensor("wgT", [3, N], f32, kind="ExternalInput")
    bwT_d = nc.dram_tensor("bwT", [3, N], f32, kind="ExternalInput")
    swT_d = nc.dram_tensor("swT", [3, N], f32, kind="ExternalInput")
    woP_d = nc.dram_tensor("woP", [N, PC], f32, kind="ExternalInput")
    coP_d = nc.dram_tensor("coP", [N, PC], f32, kind="ExternalInput")
    ooP_d = nc.dram_tensor("ooP", [N, PC], f32, kind="ExternalInput")
    smlI_d = nc.dram_tensor("smlI", [128, 32], f32, kind="ExternalInput")
    maskO_d = nc.dram_tensor("maskO", [N, PC * S], f32, kind="ExternalOutput")
    smlO_d = nc.dram_tensor("smlO", [128, 32], f32, kind="ExternalOutput")

    with tile.TileContext(nc) as tc:
        from contextlib import ExitStack
        with ExitStack() as ctx:
            cpool = ctx.enter_context(tc.tile_pool(name="const", bufs=1))
            dram = ctx.enter_context(tc.tile_pool(name="dram", bufs=1, space="DRAM"))
            psum = ctx.enter_context(tc.tile_pool(name="psum", bufs=8, space="PSUM"))
            natp = ctx.enter_context(tc.tile_pool(name="nat", bufs=2))
            outp = ctx.enter_context(tc.tile_pool(name="out", bufs=2))

            # ---- constants into SBUF ----
            dstk_t = cpool.tile([3, PC * S], f32)
            nc.sync.dma_start(dstk_t[:], dstk_d.ap())
            nrmT_t = cpool.tile([3, N], f32)
            nc.sync.dma_start(nrmT_t[:], nrmT_d.ap())
            wgT_t = cpool.tile([3, N], f32)
            nc.sync.dma_start(wgT_t[:], wgT_d.ap())
            bwT_t = cpool.tile([3, N], f32)
            nc.sync.dma_start(bwT_t[:], bwT_d.ap())
            swT_t = cpool.tile([3, N], f32)
            nc.sync.dma_start(swT_t[:], swT_d.ap())
            # [N, PC] tables -> SBUF [128, FT*PC] (face-tile along free)
            num_t = cpool.tile([128, FT * PC], f32)
            nc.sync.dma_start(num_t[:].rearrange("p (f c) -> p f c", f=FT),
                              numt_d.ap().rearrange("(f p) c -> p f c", f=FT))
            wo_t = cpool.tile([128, FT * PC], f32)
            nc.sync.dma_start(wo_t[:].rearrange("p (f c) -> p f c", f=FT),
                              woP_d.ap().rearrange("(f p) c -> p f c", f=FT))
            co_t = cpool.tile([128, FT * PC], f32)
            nc.sync.dma_start(co_t[:].rearrange("p (f c) -> p f c", f=FT),
                              coP_d.ap().rearrange("(f p) c -> p f c", f=FT))
            oo_t = cpool.tile([128, FT * PC], f32)
            nc.sync.dma_start(oo_t[:].rearrange("p (f c) -> p f c", f=FT),
                              ooP_d.ap().rearrange("(f p) c -> p f c", f=FT))

            # small passthrough (col/opa/reflected_ray)
            sml_t = cpool.tile([128, 32], f32)
            nc.sync.dma_start(sml_t[:], smlI_d.ap())
            nc.sync.dma_start(smlO_d.ap(), sml_t[:])

            # DRAM scratch: tbuf[pp, ss*N + n']  (flat-j order per point)
            tbuf = dram.tile([PC, S * N], f32)
            # natural-order write view: [n', pp, ss]
            tb_nat = tbuf[:].rearrange("p (s n) -> n p s", s=S)
            # output-order read view: [nn, pp, ss]
            tb_out = tbuf[:].rearrange("p (n s) -> n p s", s=S)

            # ---- natural phase: t = num * recip(nrm . d) ----
            for i in range(FT):
                tnat = natp.tile([128, PC * S], f32, tag="tnat")
                for j in range(SUB):
                    vd = psum.tile([128, W], f32, tag="ps")
                    nc.tensor.matmul(vd[:], nrmT_t[:, i * 128:(i + 1) * 128],
                                     dstk_t[:, j * W:(j + 1) * W],
                                     start=True, stop=True)
                    rv = natp.tile([128, W], f32, tag="rv")
                    scr = natp.tile([128, W], f32, tag="scr")
                    nc.vector.reciprocal_approx_accurate(rv[:], vd[:], scr[:])
                    # num broadcast over ss: subchunk j covers pp [j*16,(j+1)*16)
                    nb = num_t[:, i * PC + j * (W // S): i * PC + (j + 1) * (W // S)]
                    nc.gpsimd.tensor_tensor(
                        tnat[:, j * W:(j + 1) * W].rearrange("p (a s) -> p a s", s=S),
                        nb.unsqueeze(2).broadcast_to([128, W // S, S]),
                        rv[:].rearrange("p (a s) -> p a s", s=S),
                        Alu.mult)
                nc.sync.dma_start(
                    tb_nat[i * 128:(i + 1) * 128],
                    tnat[:].rearrange("p (a s) -> p a s", s=S))

            # ---- output phase: per face-tile decision chain ----
            for f in range(FT):
                t_t = outp.tile([128, PC * S], f32, tag="t")
                nc.sync.dma_start(t_t[:].rearrange("p (a s) -> p a s", s=S),
                                  tb_out[f * 128:(f + 1) * 128])

                planes = []
                for name, lhsT in (("ga", wgT_t), ("be", bwT_t), ("sv", swT_t)):
                    acc = outp.tile([128, PC * S], f32, tag=name)
                    for j in range(SUB):
                        pl = psum.tile([128, W], f32, tag="ps")
                        nc.tensor.matmul(pl[:], lhsT[:, f * 128:(f + 1) * 128],
                                         dstk_t[:, j * W:(j + 1) * W],
                                         start=True, stop=True)
                        nc.vector.tensor_tensor(acc[:, j * W:(j + 1) * W],
                                                t_t[:, j * W:(j + 1) * W],
                                                pl[:], Alu.mult)
                    planes.append(acc)
                ga_t, be_t, sv_t = planes

                def bc(tab):
                    return tab[:, f * PC:(f + 1) * PC].unsqueeze(2) \
                              .broadcast_to([128, PC, S])

                def v3(t):
                    return t[:].rearrange("p (a s) -> p a s", s=S)

                # in-place bc-adds: ga/be/sv become gam/beta/s''
                nc.gpsimd.tensor_tensor(v3(ga_t), v3(ga_t), bc(wo_t), Alu.add)
                nc.gpsimd.tensor_tensor(v3(be_t), v3(be_t), bc(co_t), Alu.add)
                nc.gpsimd.tensor_tensor(v3(sv_t), v3(sv_t), bc(oo_t), Alu.add)

                tcm = outp.tile([128, PC * S], f32, tag="tcm")
                nc.scalar.activation(tcm[:], t_t[:], Act.Copy,
                                     bias=1.0, scale=-1.0)
                # q-chain ping-pongs through be/sv/tcm (min is DVE-only)
                nc.vector.tensor_tensor(be_t[:], ga_t[:], be_t[:], Alu.min)
                nc.vector.scalar_tensor_tensor(be_t[:], t_t[:], 1e-4, be_t[:],
                                               Alu.add, Alu.min)
                nc.vector.tensor_tensor(sv_t[:], be_t[:], sv_t[:], Alu.min)
                nc.vector.tensor_tensor(sv_t[:], sv_t[:], tcm[:], Alu.min)
                nc.vector.tensor_single_scalar(tcm[:], sv_t[:], 0.0, Alu.is_gt)
                nc.sync.dma_start(maskO_d.ap()[f * 128:(f + 1) * 128, :], tcm[:])

    nc.compile()
    return nc


def _host_prep(V, indices, pointindex, COL, OPA, p, l, normals, it, hemi_vecs):
    """All the small per-point / per-face tables, fp64 where it helps."""
    f32 = np.float32
    V64 = V.astype(np.float64)
    p64 = p.astype(np.float64)
    l64 = l.astype(np.float64)
    h64 = hemi_vecs.astype(np.float64)
    idx = indices.astype(np.int64)
    pix = pointindex.astype(np.int64)

    # Rodrigues rotation -> ray directions d[pp, ss, 3]
    u = l64[None, :] - p64
    u_hat = u / np.linalg.norm(u, axis=1, keepdims=True)
    c = -u_hat[:, 1:2]
    v_loc = np.broadcast_to(np.array([0.0, -1.0, 0.0]), u_hat.shape)
    w = np.cross(v_loc, u_hat)
    z0 = np.zeros(P)
    vmat = np.stack([np.stack([z0, -w[:, 2], w[:, 1]], -1),
                     np.stack([w[:, 2], z0, -w[:, 0]], -1),
                     np.stack([-w[:, 1], w[:, 0], z0], -1)], axis=1)
    R = np.eye(3)[None] + vmat + np.matmul(vmat, vmat) / (1.0 + c)[..., None]
    lh = np.einsum('pij,sj->psi', R, h64) + l64
    d = (lh - p64[:, None, :]).astype(f32)        # [P,S,3]
    o32 = p.astype(f32)                            # [P,3]

    # plane normals / offsets
    nrm = np.cross(V64[:, 1] - V64[:, 0], V64[:, 2] - V64[:, 0])
    nrm = nrm / np.linalg.norm(nrm, axis=1, keepdims=True)
    kk = -np.sum(nrm * V64[:, 3], axis=1)
    nrm32, kk32 = nrm.astype(f32), kk.astype(f32)

    # num[pp, n'] = -(kk + o.nrm), fp32 like the reference
    vo = o32 @ nrm32.T
    numt = -(kk32[None, :] + vo)                   # [P,N]

    # per-face folded weight triples (fp64)
    a0, a1, a2 = V64[:, 0, 0], V64[:, 0, 1], V64[:, 0, 2]
    b0, b1, b2 = V64[:, 1, 0], V64[:, 1, 1], V64[:, 1, 2]
    c0, c1, c2 = V64[:, 2, 0], V64[:, 2, 1], V64[:, 2, 2]
    B = a0 * b2 - a2 * b0
    D = a0 * b1 - a1 * b0
    E = a0 * c2 - a2 * c0
    K1 = a1 * c0 - a0 * c1
    F = B * K1
    invden = 1.0 / (E * D + F)
    invD = 1.0 / D
    w0 = (B * a1 - D * a2) * invden
    w1 = (-B * a0) * invden
    w2 = (D * a0) * invden
    wg = np.stack([w0, w1, w2])                    # [3,N] gam weights
    bw = np.stack([-a1 * invD + K1 * invD * w0,
                   a0 * invD + K1 * invD * w1,
                   K1 * invD * w2])                # beta weights
    sgn = np.sign(a0)
    sw = np.stack([sgn * (1.0 - b0 * bw[0] - c0 * w0),
                   sgn * (-b0 * bw[1] - c0 * w1),
                   sgn * (-b0 * bw[2] - c0 * w2)])  # s'' weights

    # broadcast (o-dot) planes [N, P]
    woP = (wg.T @ p64.T)                           # wg_k[n]*o_k[pp]
    coP = (bw.T @ p64.T)
    ooP = (sw.T @ p64.T)

    # empty fold: gam plane gets -1e30 where (pp, face) is masked out
    local = pix % P
    surf = idx[pix, 0]
    mat = idx[pix, 1]
    empty = np.zeros((P, N), bool)
    empty[local, surf] = True
    woP = woP.astype(f32)
    woP[empty.T] = NEG_BIG
    coP, ooP = coP.astype(f32), ooP.astype(f32)

    # small outputs
    col = COL[surf, mat]                           # [P,3] f32
    opa = np.clip(OPA[surf, mat], 0.0, 1.0)
    refl = (l[None, :].astype(f32) - p.astype(f32))
    sml = np.zeros((P, 8), f32)
    sml[:, 0:3] = col
    sml[:, 3] = opa
    sml[:, 4:7] = refl
    smlI = sml.reshape(128, 32)

    # device input stacks
    dstk = np.ascontiguousarray(
        d.transpose(2, 0, 1).reshape(3, P * S))    # [3, pp*S+ss]
    nrmT = np.ascontiguousarray(nrm32.T)           # [3,N]
    return dict(dstk=dstk, nrmT=nrmT, numt=np.ascontiguousarray(numt.T),
                wg=wg.astype(f32), bw=bw.astype(f32), sw=sw.astype(f32),
                woP=woP, coP=coP, ooP=ooP, smlI=smlI,
                col=col, opa=opa, refl=refl)


def kernel(V, indices, pointindex, COL, OPA, p, l, normals, it, hemi_vecs):
    from concourse import bass_utils

    V = np.asarray(V); COL = np.asarray(COL); OPA = np.asarray(OPA)
    p = np.asarray(p); l = np.asarray(l)
    hemi_vecs = np.asarray(hemi_vecs)
    indices = np.asarray(indices); pointindex = np.asarray(pointindex)

    h = _host_prep(V, indices, pointindex, COL, OPA, p, l,
                   np.asarray(normals), it, hemi_vecs)

    if "nc" not in _cache:
        _cache["nc"] = _build_module()
    nc = _cache["nc"]

    in_maps = []
    for k in range(NCORES):
        sl = slice(k * PC, (k + 1) * PC)
        in_maps.append({
            "dstk": np.ascontiguousarray(
                h["dstk"][:, k * PC * S:(k + 1) * PC * S]),
            "nrmT": h["nrmT"],
            "numt": np.ascontiguousarray(h["numt"][:, sl]),
            "wgT": h["wg"], "bwT": h["bw"], "swT": h["sw"],
            "woP": np.ascontiguousarray(h["woP"][:, sl]),
            "coP": np.ascontiguousarray(h["coP"][:, sl]),
            "ooP": np.ascontiguousarray(h["ooP"][:, sl]),
            "smlI": h["smlI"],
        })

    res = bass_utils.run_bass_kernel_spmd(nc, in_maps,
                                          core_ids=list(range(NCORES)))
    _cache["last_results"] = res
    outs = res.results

    mask = np.stack([outs[k]["maskO"].reshape(N, PC, S)
                     for k in range(NCORES)])      # [8, N, PC, S]
    mask = np.ascontiguousarray(
        mask.transpose(0, 2, 1, 3).reshape(P, N, S))

    sml = outs[0]["smlO"].reshape(P, 8)
    col = np.ascontiguousarray(sml[:, 0:3])
    opa = np.ascontiguousarray(sml[:, 3])
    refl = np.ascontiguousarray(sml[:, 4:7])
    return mask, col, opa, refl


# revision 6
# speedup vs baseline: 15.0591x; 15.0591x over previous
"""Trainium2 Bass kernel for nn_DiffuseShader.

Math restructuring (validated against the jax reference to ~1 flip in 13.1M
mask elements):

The reference computes, per point pp (P=512), face nn (N=1024), ray ss (S=25):
  t        ray/plane distance, but read through a torch-style .view(P,N,S) of
           the natural [P*S, N] buffer -> t[pp,nn,ss] = the flat per-point
           buffer at j = nn*S+ss, i.e. tnat[pp, j//N, j%N].
  gam/beta/alpha   barycentric quantities, each a LINEAR functional of
           r = o + t*d:  gam = wg.r, beta = bw.r, s'' = sw.r  (per-face fp64
           folded weights, with invden/invD/sign folded in).
  mask = [gam>0 & beta>0 & s''>0 & t>-1e-4 & t<1 & ~empty]  as 0.0/1.0.

Sharding: points across the 8 cores (64 each), embarrassingly parallel.

Per core device pipeline (free order is always (ray, point) = (s, pp), pp
innermost, so every DMA touching the DRAM scratch moves contiguous runs):
  natural phase:  vd[n', (s,pp)] = nrm.d  via PE (K=3 matmuls),
                  t = num * 1/vd (custom-DVE approx reciprocal, ~2 ULP),
                  DMA to DRAM scratch tbuf[j, pp] with j = s*N+n'
                  (25 runs of 256B per partition).
  output phase:   re-read tbuf rows j = nn*25+ss for face-tile nn — that IS
                  the .view scramble, and in this layout it's a fully
                  contiguous 6.4KB read per partition.  PE computes the three
                  weight planes (weights x d), DVE/GPSIMD/ACT run the
                  decision chain, mask written as [nn, (ss,pp)] — the host
                  gather transposes to [pp,nn,ss].
"""
import numpy as np

P, N, S, M = 512, 1024, 25, 8
NCORES = 8
PC = P // NCORES          # 64 points per core
FT = N // 128             # 8 face tiles
SUB = 4                   # 400-wide psum subchunks per 1600-wide chunk
W = PC * S // SUB         # 400
NEG_BIG = np.float32(-1e30)

_cache = {}


def _build_module():
    import concourse.bass as bass
    import concourse.tile as tile
    from concourse import bacc, mybir

    f32 = mybir.dt.float32
    Alu = mybir.AluOpType
    Act = mybir.ActivationFunctionType

    nc = bacc.Bacc("TRN2", target_bir_lowering=False, debug=False,
                   num_devices=NCORES)

    dstk_d = nc.dram_tensor("dstk", [3, S * PC], f32, kind="ExternalInput")
    nrmT_d = nc.dram_tensor("nrmT", [3, N], f32, kind="ExternalInput")
    numt_d = nc.dram_tensor("numt", [N, PC], f32, kind="ExternalInput")
    wgT_d = nc.dram_tensor("wgT", [3, N], f32, kind="ExternalInput")
    bwT_d = nc.dram_tensor("bwT", [3, N], f32, kind="ExternalInput")
    swT_d = nc.dram_tensor("swT", [3, N], f32, kind="ExternalInput")
    woP_d = nc.dram_tensor("woP", [N, PC], f32, kind="ExternalInput")
    coP_d = nc.dram_tensor("coP", [N, PC], f32, kind="ExternalInput")
    ooP_d = nc.dram_tensor("ooP", [N, PC], f32, kind="ExternalInput")
    smlI_d = nc.dram_tensor("smlI", [128, 32], f32, kind="ExternalInput")
    maskO_d = nc.dram_tensor("maskO", [N, S * PC], f32, kind="ExternalOutput")
    smlO_d = nc.dram_tensor("smlO", [128, 32], f32, kind="ExternalOutput")

    with tile.TileContext(nc) as tc:
        from contextlib import ExitStack
        with ExitStack() as ctx:
            cpool = ctx.enter_context(tc.tile_pool(name="const", bufs=1))
            dram = ctx.enter_context(tc.tile_pool(name="dram", bufs=1, space="DRAM"))
            psum = ctx.enter_context(tc.tile_pool(name="psum", bufs=8, space="PSUM"))
            natp = ctx.enter_context(tc.tile_pool(name="nat", bufs=2))
            outp = ctx.enter_context(tc.tile_pool(name="out", bufs=2))

            # ---- constants into SBUF ----
            dstk_t = cpool.tile([3, S * PC], f32)
            nc.sync.dma_start(dstk_t[:], dstk_d.ap())
            nrmT_t = cpool.tile([3, N], f32)
            nc.sync.dma_start(nrmT_t[:], nrmT_d.ap())
            wgT_t = cpool.tile([3, N], f32)
            nc.sync.dma_start(wgT_t[:], wgT_d.ap())
            bwT_t = cpool.tile([3, N], f32)
            nc.sync.dma_start(bwT_t[:], bwT_d.ap())
            swT_t = cpool.tile([3, N], f32)
            nc.sync.dma_start(swT_t[:], swT_d.ap())
            # [N, PC] tables -> SBUF [128, FT*PC] (face-tile along free)
            num_t = cpool.tile([128, FT * PC], f32)
            nc.sync.dma_start(num_t[:].rearrange("p (f c) -> p f c", f=FT),
                              numt_d.ap().rearrange("(f p) c -> p f c", f=FT))
            wo_t = cpool.tile([128, FT * PC], f32)
            nc.sync.dma_start(wo_t[:].rearrange("p (f c) -> p f c", f=FT),
                              woP_d.ap().rearrange("(f p) c -> p f c", f=FT))
            co_t = cpool.tile([128, FT * PC], f32)
            nc.sync.dma_start(co_t[:].rearrange("p (f c) -> p f c", f=FT),
                              coP_d.ap().rearrange("(f p) c -> p f c", f=FT))
            oo_t = cpool.tile([128, FT * PC], f32)
            nc.sync.dma_start(oo_t[:].rearrange("p (f c) -> p f c", f=FT),
                              ooP_d.ap().rearrange("(f p) c -> p f c", f=FT))

            # small passthrough (col/opa/reflected_ray)
            sml_t = cpool.tile([128, 32], f32)
            nc.sync.dma_start(sml_t[:], smlI_d.ap())
            nc.sync.dma_start(smlO_d.ap(), sml_t[:])

            # DRAM scratch: tbuf[j, pp] with j = s*N + n'  (pp contiguous)
            tbuf = dram.tile([S * N, PC], f32)
            # natural-order write view: [n', s, pp]
            tb_nat = tbuf[:].rearrange("(s n) c -> n s c", n=N)
            # output-order read view: [nn, (ss,pp)] — contiguous rows
            tb_out = tbuf[:].rearrange("(n s) c -> n (s c)", s=S)

            # ---- natural phase: t = num * recip(nrm . d) ----
            for i in range(FT):
                tnat = natp.tile([128, S * PC], f32, tag="tnat")
                rv = natp.tile([128, S * PC], f32, tag="rv")
                scr = natp.tile([128, W], f32, tag="scr", bufs=2)
                for j in range(SUB):
                    vd = psum.tile([128, W], f32, tag="ps")
                    nc.tensor.matmul(vd[:], nrmT_t[:, i * 128:(i + 1) * 128],
                                     dstk_t[:, j * W:(j + 1) * W],
                                     start=True, stop=True)
                    nc.vector.reciprocal_approx_accurate(
                        rv[:, j * W:(j + 1) * W], vd[:], scr[:])
                # num broadcast over s (outer free dim)
                nb = num_t[:, i * PC:(i + 1) * PC]
                nc.gpsimd.tensor_tensor(
                    tnat[:].rearrange("p (s a) -> p s a", a=PC),
                    nb.unsqueeze(1).broadcast_to([128, S, PC]),
                    rv[:].rearrange("p (s a) -> p s a", a=PC),
                    Alu.mult)
                nc.sync.dma_start(
                    tb_nat[i * 128:(i + 1) * 128],
                    tnat[:].rearrange("p (s a) -> p s a", a=PC))

            # ---- output phase: per face-tile decision chain ----
            for f in range(FT):
                t_t = outp.tile([128, S * PC], f32, tag="t")
                nc.sync.dma_start(t_t[:], tb_out[f * 128:(f + 1) * 128])

                planes = []
                for name, lhsT in (("ga", wgT_t), ("be", bwT_t), ("sv", swT_t)):
                    acc = outp.tile([128, S * PC], f32, tag=name)
                    for j in range(SUB):
                        pl = psum.tile([128, W], f32, tag="ps")
                        nc.tensor.matmul(pl[:], lhsT[:, f * 128:(f + 1) * 128],
                                         dstk_t[:, j * W:(j + 1) * W],
                                         start=True, stop=True)
                        nc.vector.tensor_tensor(acc[:, j * W:(j + 1) * W],
                                                t_t[:, j * W:(j + 1) * W],
                                                pl[:], Alu.mult)
                    planes.append(acc)
                ga_t, be_t, sv_t = planes

                def bc(tab):
                    return tab[:, f * PC:(f + 1) * PC].unsqueeze(1) \
                              .broadcast_to([128, S, PC])

                def v3(t):
                    return t[:].rearrange("p (s a) -> p s a", a=PC)

                # in-place bc-adds: ga/be/sv become gam/beta/s''
                nc.gpsimd.tensor_tensor(v3(ga_t), v3(ga_t), bc(wo_t), Alu.add)
                nc.gpsimd.tensor_tensor(v3(be_t), v3(be_t), bc(co_t), Alu.add)
                nc.gpsimd.tensor_tensor(v3(sv_t), v3(sv_t), bc(oo_t), Alu.add)

                tcm = outp.tile([128, S * PC], f32, tag="tcm")
                nc.scalar.activation(tcm[:], t_t[:], Act.Copy,
                                     bias=1.0, scale=-1.0)
                # q-chain ping-pongs through be/sv/tcm (min is DVE-only)
                nc.vector.tensor_tensor(be_t[:], ga_t[:], be_t[:], Alu.min)
                nc.vector.scalar_tensor_tensor(be_t[:], t_t[:], 1e-4, be_t[:],
                                               Alu.add, Alu.min)
                nc.vector.tensor_tensor(sv_t[:], be_t[:], sv_t[:], Alu.min)
                nc.vector.tensor_tensor(sv_t[:], sv_t[:], tcm[:], Alu.min)
                nc.vector.tensor_single_scalar(tcm[:], sv_t[:], 0.0, Alu.is_gt)
                nc.sync.dma_start(maskO_d.ap()[f * 128:(f + 1) * 128, :], tcm[:])

    nc.compile()
    return nc


def _host_prep(V, indices, pointindex, COL, OPA, p, l, normals, it, hemi_vecs):
    """All the small per-point / per-face tables, fp64 where it helps."""
    f32 = np.float32
    V64 = V.astype(np.float64)
    p64 = p.astype(np.float64)
    l64 = l.astype(np.float64)
    h64 = hemi_vecs.astype(np.float64)
    idx = indices.astype(np.int64)
    pix = pointindex.astype(np.int64)

    # Rodrigues rotation -> ray directions d[pp, ss, 3]
    u = l64[None, :] - p64
    u_hat = u / np.linalg.norm(u, axis=1, keepdims=True)
    c = -u_hat[:, 1:2]
    v_loc = np.broadcast_to(np.array([0.0, -1.0, 0.0]), u_hat.shape)
    w = np.cross(v_loc, u_hat)
    z0 = np.zeros(P)
    vmat = np.stack([np.stack([z0, -w[:, 2], w[:, 1]], -1),
                     np.stack([w[:, 2], z0, -w[:, 0]], -1),
                     np.stack([-w[:, 1], w[:, 0], z0], -1)], axis=1)
    R = np.eye(3)[None] + vmat + np.matmul(vmat, vmat) / (1.0 + c)[..., None]
    lh = np.einsum('pij,sj->psi', R, h64) + l64
    d = (lh - p64[:, None, :]).astype(f32)        # [P,S,3]
    o32 = p.astype(f32)                            # [P,3]

    # plane normals / offsets
    nrm = np.cross(V64[:, 1] - V64[:, 0], V64[:, 2] - V64[:, 0])
    nrm = nrm / np.linalg.norm(nrm, axis=1, keepdims=True)
    kk = -np.sum(nrm * V64[:, 3], axis=1)
    nrm32, kk32 = nrm.astype(f32), kk.astype(f32)

    # num[pp, n'] = -(kk + o.nrm), fp32 like the reference
    vo = o32 @ nrm32.T
    numt = -(kk32[None, :] + vo)                   # [P,N]

    # per-face folded weight triples (fp64)
    a0, a1, a2 = V64[:, 0, 0], V64[:, 0, 1], V64[:, 0, 2]
    b0, b1, b2 = V64[:, 1, 0], V64[:, 1, 1], V64[:, 1, 2]
    c0, c1, c2 = V64[:, 2, 0], V64[:, 2, 1], V64[:, 2, 2]
    B = a0 * b2 - a2 * b0
    D = a0 * b1 - a1 * b0
    E = a0 * c2 - a2 * c0
    K1 = a1 * c0 - a0 * c1
    F = B * K1
    invden = 1.0 / (E * D + F)
    invD = 1.0 / D
    w0 = (B * a1 - D * a2) * invden
    w1 = (-B * a0) * invden
    w2 = (D * a0) * invden
    wg = np.stack([w0, w1, w2])                    # [3,N] gam weights
    bw = np.stack([-a1 * invD + K1 * invD * w0,
                   a0 * invD + K1 * invD * w1,
                   K1 * invD * w2])                # beta weights
    sgn = np.sign(a0)
    sw = np.stack([sgn * (1.0 - b0 * bw[0] - c0 * w0),
                   sgn * (-b0 * bw[1] - c0 * w1),
                   sgn * (-b0 * bw[2] - c0 * w2)])  # s'' weights

    # broadcast (o-dot) planes [N, P]
    woP = (wg.T @ p64.T)                           # wg_k[n]*o_k[pp]
    coP = (bw.T @ p64.T)
    ooP = (sw.T @ p64.T)

    # empty fold: gam plane gets -1e30 where (pp, face) is masked out
    local = pix % P
    surf = idx[pix, 0]
    mat = idx[pix, 1]
    empty = np.zeros((P, N), bool)
    empty[local, surf] = True
    woP = woP.astype(f32)
    woP[empty.T] = NEG_BIG
    coP, ooP = coP.astype(f32), ooP.astype(f32)

    # small outputs
    col = COL[surf, mat]                           # [P,3] f32
    opa = np.clip(OPA[surf, mat], 0.0, 1.0)
    refl = (l[None, :].astype(f32) - p.astype(f32))
    sml = np.zeros((P, 8), f32)
    sml[:, 0:3] = col
    sml[:, 3] = opa
    sml[:, 4:7] = refl
    smlI = sml.reshape(128, 32)

    # device input stacks: dstk[k, s*PC+pp] per core (s outer, pp inner)
    dstk = np.ascontiguousarray(d.transpose(2, 1, 0))   # [3, S, P]
    nrmT = np.ascontiguousarray(nrm32.T)                # [3,N]
    return dict(dstk=dstk, nrmT=nrmT, numt=np.ascontiguousarray(numt.T),
                wg=wg.astype(f32), bw=bw.astype(f32), sw=sw.astype(f32),
                woP=woP, coP=coP, ooP=ooP, smlI=smlI,
                col=col, opa=opa, refl=refl)


def kernel(V, indices, pointindex, COL, OPA, p, l, normals, it, hemi_vecs):
    from concourse import bass_utils

    V = np.asarray(V); COL = np.asarray(COL); OPA = np.asarray(OPA)
    p = np.asarray(p); l = np.asarray(l)
    hemi_vecs = np.asarray(hemi_vecs)
    indices = np.asarray(indices); pointindex = np.asarray(pointindex)

    h = _host_prep(V, indices, pointindex, COL, OPA, p, l,
                   np.asarray(normals), it, hemi_vecs)

    if "nc" not in _cache:
        _cache["nc"] = _build_module()
    nc = _cache["nc"]

    in_maps = []
    for k in range(NCORES):
        sl = slice(k * PC, (k + 1) * PC)
        in_maps.append({
            "dstk": np.ascontiguousarray(
                h["dstk"][:, :, sl].reshape(3, S * PC)),
            "nrmT": h["nrmT"],
            "numt": np.ascontiguousarray(h["numt"][:, sl]),
            "wgT": h["wg"], "bwT": h["bw"], "swT": h["sw"],
            "woP": np.ascontiguousarray(h["woP"][:, sl]),
            "coP": np.ascontiguousarray(h["coP"][:, sl]),
            "ooP": np.ascontiguousarray(h["ooP"][:, sl]),
            "smlI": h["smlI"],
        })

    res = bass_utils.run_bass_kernel_spmd(nc, in_maps,
                                          core_ids=list(range(NCORES)))
    _cache["last_results"] = res
    outs = res.results

    # maskO[nn, ss*PC+pp] -> full [P, N, S]
    mask = np.stack([outs[k]["maskO"].reshape(N, S, PC)
                     for k in range(NCORES)])      # [8, N, S, PC]
    mask = np.ascontiguousarray(
        mask.transpose(0, 3, 1, 2).reshape(P, N, S))

    sml = outs[0]["smlO"].reshape(P, 8)
    col = np.ascontiguousarray(sml[:, 0:3])
    opa = np.ascontiguousarray(sml[:, 3])
    refl = np.ascontiguousarray(sml[:, 4:7])
    return mask, col, opa, refl
